# revision 27
# baseline (speedup 1.0000x reference)
"""NeighborRoutingConv (GAT-style multi-head edge-softmax message passing) on 8 trn2 cores.

Strategy (v7, uniform-slot + engine-balanced edition; evolved from v6):
  - Host change of basis per head: T_k (row 0 == attn_k) folds the attention
    logit INTO the message vector: V[n,k,:] = T_k @ Wh[n,k,:], a[n,k] =
    V[n,k,0]; host applies T_k^{-1} to the aggregated output.  V columns
    (d,k)-interleaved (k minor) keep unit-stride last dims on DVE.
  - Host NODE PERMUTATION: nodes are packed into 400 dst bins of 128
    positions such that every bin has <=1024 in-edges from EACH src half
    (2D greedy bin-packing on (deg_from_A, deg_from_B); halves balanced on
    total degree first, each half exactly 200 bins so no bin straddles).
    Every (core, slot) processes exactly one bin with a UNIFORM 8+8 chunk
    shape (1024 gather idx per segment) -> ~2.4% gather padding and a single
    compile-time shape; cores take contiguous 50-bin ranges.
  - Per-core COMPACTED table: each core's hT input holds only the ~44k
    permuted columns whose V rows it will actually gather (its edges'
    sources + its own dst rows), padded per half to LHALF=23040 so the
    program stays SPMD-uniform -> phase 1 shrinks ~10%.
  - Phase 1 (bf16 matmuls, PE-bound ~79us): whaug[n] = V row (512B) into
    core-local DRAM.  hT read on SP, table write on Pool, PSUM->SBUF copies
    alternate Act/DVE; 2-bank PSUM tiles x4 bufs keep the mm->copy->write
    chain fully pipelined.  NO barrier into phase 2: the table writes and
    all gathers share Pool's in-order queue, so write->gather ordering is
    automatic and the phases overlap; Pool runs a dense writes-then-gathers
    stream at ~93% occupancy, which IS the kernel's critical path (writes
    must stay on Pool: offloading them leaves Pool idling for the B-half
    table instead, SBUF cannot hold enough prefetched A-segments to bridge,
    and an fp8 table loses the bf16 a_src precision the softmax needs).
  - Phase 2 (one bin per slot, 50 slots/core; Pool/gather-bound ~183us),
    4-deep software-pipelined stages so every engine's in-order queue sees
    only satisfied deps:
      s0: two 1024-idx 512B-row dma_gathers (Pool, the serial byte-bound
          resource) + sel one-hot loaded via plain DMA on the otherwise-idle
          SP stream (host precomputes sel; saves 55us of DVE is_equal)
      s1: per-chunk PE transposes (batched x8 into bf16 PSUM) + Act copy to
          SBUF + per-chunk a_dst scatter matmuls vs ad_all (a_dst rows
          JIT-gathered per 8-slot group; wrong-half idx aim at a zero pad
          row so A/B merge is one add)
      s2: s_t = a_src + a_dst (DVE) -> leaky (DVE) -> exp (Act; Lrelu on Act
          would force a 1.3us act-table reload per switch) -> alpha-multiply
          (DVE 2x, bcast over DK)
      s3: segment_sum(msgs) + segment_sum(e_exp) via PE PSUM accumulation
      s4: reciprocal (DVE) + PSUM evacuation (Act) + bf16 2x normalize into
          a resident [P, J, 256] output tile, written out in 8-slot chunks
  Softmax max-subtraction skipped (|a| <~ 10, fp32 exp range is fine).
  Cost-model: 385.6us (v6) -> 274.4us (-28.8%).
"""

from contextlib import ExitStack

import numpy as np
import ml_dtypes

BF16 = ml_dtypes.bfloat16

P = 128
IN_DIM = 256
OUT_DIM = 256
K = 8
DK = 32
ROW = 256  # whaug row stride (bf16) = 512B
NEG_SLOPE = 0.2
N_CORES = 8
SUPER = 4  # node tiles per phase-1 iteration (512 nodes)
TGRP = 8  # sel-transposes batched per PSUM tile / Act copy
BLK = 128  # dst nodes per bin/block
NBINS = 400
N_PAD = NBINS * BLK  # 51200
HALF = N_PAD // 2  # 25600 (= bins 0..199 exactly; no bin straddles)
J = NBINS // N_CORES  # 50 slots per core
JP = 56  # J rounded up to 8 (a_dst gather grouping)
CPB = 8  # chunks per segment (uniform)
NCH = 2 * CPB  # 16 chunks per slot
SEGCAP = CPB * P  # 1024 edges per (bin, segment) hard cap
# per-core COMPACTED table: each core's whaug holds only the rows it will
# actually gather (its edges' sources + its own dst rows + the zero rows),
# padded per half to a fixed size so the program stays SPMD-uniform
LHALF = 23040  # compacted rows per half (max observed need is ~22.7k)
N_TAB = 2 * LHALF  # 46080 = 90 * 512


def _wrap16(lst):
    """dma_gather idx layout: [128, len//16] int16; idx i at [i%16, i//16],
    replicated across the 8 groups of 16 partitions."""
    n = len(lst)
    assert n % 16 == 0
    base = np.asarray(lst, dtype=np.int16).reshape(n // 16, 16).T  # [16, cols]
    return np.tile(base, (8, 1))  # [128, cols]


def _pack_half(nodes, dA, dB, nbins):
    """Greedy 2D bin-packing: place `nodes` (desc by total degree) into
    `nbins` bins with per-bin caps sumA<=SEGCAP, sumB<=SEGCAP, count<=BLK.
    Returns list of node-lists."""
    sA = np.zeros(nbins, dtype=np.int64)
    sB = np.zeros(nbins, dtype=np.int64)
    cnt = np.zeros(nbins, dtype=np.int64)
    bins = [[] for _ in range(nbins)]
    order = nodes[np.argsort(-(dA[nodes] + dB[nodes]), kind="stable")]
    for n in order:
        a, b = dA[n], dB[n]
        feas = (sA + a <= SEGCAP) & (sB + b <= SEGCAP) & (cnt < BLK)
        if not feas.any():
            raise RuntimeError("bin packing infeasible")
        score = np.maximum(sA + a, sB + b).astype(np.float64)
        score[~feas] = np.inf
        i = int(np.argmin(score))
        bins[i].append(int(n))
        sA[i] += a
        sB[i] += b
        cnt[i] += 1
    return bins


def build_plan(edge_src, edge_dst, n_nodes, n_cores):
    edge_src = np.asarray(edge_src, dtype=np.int64)
    edge_dst = np.asarray(edge_dst, dtype=np.int64)

    # ---- step 1: split nodes into two halves balanced on total in-degree
    deg = np.bincount(edge_dst, minlength=n_nodes)
    order = np.argsort(-deg, kind="stable")
    half_of = np.zeros(n_nodes, dtype=np.int8)
    hsum = [0, 0]
    hcnt = [0, 0]
    for n in order:
        h = 0 if (hsum[0] <= hsum[1] and hcnt[0] < HALF) or hcnt[1] >= HALF else 1
        half_of[n] = h
        hsum[h] += int(deg[n])
        hcnt[h] += 1

    # ---- step 2: per-dst in-degree split by src half
    srcA = half_of[edge_src] == 0
    dA = np.bincount(edge_dst[srcA], minlength=n_nodes)
    dB = np.bincount(edge_dst[~srcA], minlength=n_nodes)

    # ---- step 3: 2D bin-pack each half into NBINS/2 bins
    nodesA = np.nonzero(half_of == 0)[0]
    nodesB = np.nonzero(half_of == 1)[0]
    binsA = _pack_half(nodesA, dA, dB, NBINS // 2)
    binsB = _pack_half(nodesB, dA, dB, NBINS // 2)
    bins = binsA + binsB  # bins 0..199 in A half, 200..399 in B half

    # ---- positions
    posof = np.full(n_nodes, -1, dtype=np.int64)
    for b, lst in enumerate(bins):
        for i, n in enumerate(lst):
            posof[n] = b * BLK + i
    assert (posof >= 0).all()

    # ---- edges -> per (bin, segment) lists
    spos = posof[edge_src]
    dpos = posof[edge_dst]
    ebin = dpos >> 7
    eseg = (spos >= HALF).astype(np.int64)  # 0 = A, 1 = B
    key = ebin * 2 + eseg
    eord = np.argsort(key, kind="stable")
    key_s = key[eord]
    spos_s = spos[eord]
    dpos_s = dpos[eord]
    bounds = np.searchsorted(key_s, np.arange(2 * NBINS + 1))

    gA = np.zeros((n_cores, P, J * SEGCAP // 16), dtype=np.int16)
    gB = np.zeros((n_cores, P, J * SEGCAP // 16), dtype=np.int16)
    dcol = np.full((n_cores, P, J * NCH), -1, dtype=np.int64)
    adA = np.zeros((n_cores, P, JP * 8), dtype=np.int16)
    adB = np.zeros((n_cores, P, JP * 8), dtype=np.int16)

    # one guaranteed-zero (pad) position per half: wrong-half a_dst gathers
    # read it, so the blend collapses to a single add
    cntA = [len(b) for b in bins[: NBINS // 2]]
    cntB = [len(b) for b in bins[NBINS // 2 :]]
    bA = int(np.argmin(cntA))
    bB = int(np.argmin(cntB))
    assert cntA[bA] < BLK and cntB[bB] < BLK
    zeroA = bA * BLK + cntA[bA]
    zeroB = (NBINS // 2 + bB) * BLK + cntB[bB] - HALF

    cols = np.full((n_cores, N_TAB), -1, dtype=np.int64)
    for c in range(n_cores):
        adAl = np.full(JP * P, zeroA, dtype=np.int64)
        adBl = np.full(JP * P, HALF + zeroB, dtype=np.int64)
        gAl = np.zeros(J * SEGCAP, dtype=np.int64)
        gBl = np.zeros(J * SEGCAP, dtype=np.int64)
        for j in range(J):
            b = c * J + j
            base = b * BLK
            inA = b < NBINS // 2
            rows = base + np.arange(BLK)
            if inA:
                adAl[j * P : (j + 1) * P] = rows
            else:
                adBl[j * P : (j + 1) * P] = rows
            for seg, glist, off in ((0, gAl, 0), (1, gBl, 0)):
                lo, hi = bounds[b * 2 + seg], bounds[b * 2 + seg + 1]
                n = hi - lo
                assert n <= SEGCAP
                s = spos_s[lo:hi]
                d = dpos_s[lo:hi] - base
                glist[j * SEGCAP : j * SEGCAP + n] = s
                i_ = np.arange(n)
                dcol[c, i_ & (P - 1), j * NCH + seg * CPB + (i_ >> 7)] = d
        # gAl pads are 0 (a valid A position); gBl pads must be a valid B
        # position
        gBl[gBl == 0] = HALF
        # compact: this core only ever touches these rows
        needA = np.unique(np.concatenate([gAl, adAl[adAl < HALF],
                                          np.asarray([zeroA])]))
        needB = np.unique(np.concatenate([gBl, adBl[adBl >= HALF],
                                          np.asarray([HALF + zeroB])]))
        assert needA.max() < HALF and needB.min() >= HALF
        assert len(needA) <= LHALF and len(needB) <= LHALF, (
            len(needA), len(needB))
        gA[c] = _wrap16(np.searchsorted(needA, gAl))
        gB[c] = _wrap16(np.searchsorted(needB, gBl))
        aA = np.searchsorted(needA, np.where(adAl < HALF, adAl, zeroA))
        aB = np.searchsorted(needB, np.where(adBl >= HALF, adBl,
                                             HALF + zeroB))
        adA[c] = _wrap16(aA)
        adB[c] = _wrap16(aB)
        cols[c, 0 : len(needA)] = needA
        cols[c, LHALF : LHALF + len(needB)] = needB

    # explicit one-hot sel masks, shipped as input and DMA'd per slot on the
    # otherwise-idle SP stream (replaces on-device is_equal generation)
    sel = np.zeros((n_cores, P, J, P, NCH), dtype=BF16)
    dc = dcol.reshape(n_cores, P, J, NCH)
    cc, pp, jj_, ii = np.nonzero(dc >= 0)
    sel[cc, pp, jj_, dc[cc, pp, jj_, ii], ii] = 1
    sel = sel.reshape(n_cores, P, J * P * NCH)

    return {
        "bins": bins,
        "gA": gA,
        "gB": gB,
        "sel": sel,
        "adA": adA,
        "adB": adB,
        "cols": cols,
        "posof": posof,
    }


def build_program(n_cores):
    import concourse.bass as bass
    import concourse.tile as tile
    from concourse import bacc, mybir

    f32 = mybir.dt.float32
    bf16 = mybir.dt.bfloat16
    i16 = mybir.dt.int16

    NT = N_TAB // (P * SUPER)  # 90
    CG = IN_DIM // P  # 2
    WID = OUT_DIM

    nc = bacc.Bacc("TRN2", target_bir_lowering=False, debug=False,
                   num_devices=n_cores)

    hT = nc.dram_tensor("hT", [IN_DIM, N_TAB], bf16, kind="ExternalInput")
    waugT = nc.dram_tensor("waugT", [IN_DIM, WID], bf16, kind="ExternalInput")
    gA_d = nc.dram_tensor("gA", [P, J * SEGCAP // 16], i16, kind="ExternalInput")
    gB_d = nc.dram_tensor("gB", [P, J * SEGCAP // 16], i16, kind="ExternalInput")
    adA_d = nc.dram_tensor("adA", [P, JP * 8], i16, kind="ExternalInput")
    adB_d = nc.dram_tensor("adB", [P, JP * 8], i16, kind="ExternalInput")
    sel_d = nc.dram_tensor("seld", [P, J * P * NCH], bf16, kind="ExternalInput")
    ident_d = nc.dram_tensor("ident", [P, P], bf16, kind="ExternalInput")
    out_d = nc.dram_tensor("out", [J * BLK, OUT_DIM], bf16, kind="ExternalOutput")
    whaug = nc.dram_tensor("whaug", [N_TAB, ROW], bf16)

    with tile.TileContext(nc) as tc, ExitStack() as ctx:
        consts = ctx.enter_context(tc.tile_pool(name="consts", bufs=1))
        # outsb + m0 pools open before the phase-1 pools (LIFO release order)
        outp = ctx.enter_context(tc.tile_pool(name="outp", bufs=1))
        m0p = ctx.enter_context(tc.tile_pool(name="m0p", bufs=6))
        ctx1 = ctx.enter_context(ExitStack())
        p1in = ctx1.enter_context(tc.tile_pool(name="p1in", bufs=6))
        p1ps = ctx1.enter_context(tc.tile_pool(name="p1ps", bufs=4, space="PSUM"))
        p1st = ctx1.enter_context(tc.tile_pool(name="p1st", bufs=6))

        waug_sb = consts.tile([P, CG, WID], bf16)
        nc.sync.dma_start(out=waug_sb[:],
                          in_=waugT.ap().rearrange("(g p) r -> p g r", p=P))
        ident_sb = consts.tile([P, P], bf16)
        nc.sync.dma_start(out=ident_sb[:], in_=ident_d.ap())
        gA_sb = consts.tile([P, J * SEGCAP // 16], i16)
        nc.gpsimd.dma_start(out=gA_sb[:], in_=gA_d.ap())
        gB_sb = consts.tile([P, J * SEGCAP // 16], i16)
        nc.gpsimd.dma_start(out=gB_sb[:], in_=gB_d.ap())
        adA_sb = consts.tile([P, JP * 8], i16)
        nc.scalar.dma_start(out=adA_sb[:], in_=adA_d.ap())
        adB_sb = consts.tile([P, JP * 8], i16)
        nc.scalar.dma_start(out=adB_sb[:], in_=adB_d.ap())

        outsb = outp.tile([P, J, OUT_DIM], bf16)

        # ---- phase 1 ----
        hT_r = hT.ap().rearrange("(g p) n -> p g n", p=P)
        wh_r = whaug.ap().rearrange("(i t p) r -> i p t r", t=SUPER, p=P)
        for it in range(NT):
            ht = p1in.tile([P, CG, SUPER * P], bf16)
            nc.sync.dma_start(
                out=ht[:], in_=hT_r[:, :, it * SUPER * P : (it + 1) * SUPER * P]
            )
            # 256-wide tiles: 1KB-aligned per-t windows stay in one bank,
            # tile = 2 banks so 4 PSUM bufs fit -> mm/copy fully pipelined
            ps = p1ps.tile([P, SUPER, WID], f32)
            for t in range(SUPER):
                for g in range(CG):
                    nc.tensor.matmul(
                        out=ps[:, t, :],
                        lhsT=ht[:, g, t * P : (t + 1) * P],
                        rhs=waug_sb[:, g, :],
                        start=(g == 0),
                        stop=(g == CG - 1),
                    )
            st = p1st.tile([P, SUPER, WID], bf16)
            if it % 2 == 0:
                nc.scalar.copy(out=st[:], in_=ps[:])
            else:
                nc.vector.tensor_copy(st[:], ps[:])
            nc.gpsimd.dma_start(out=wh_r[it][:, :, 0:WID], in_=st[:])

        ctx1.close()
        # no barrier: whaug writes and all gathers share Pool's in-order
        # queue, and every SBUF/PSUM dependency is tile-tracked

        # ---- phase 2 ----
        tabA = whaug.ap()[0:LHALF, :]
        tabB = whaug.ap()[LHALF:N_TAB, :]
        HR = ROW // 2  # 128 bf16 = 256B elems for the a_dst gathers
        tabAt = whaug.ap()[0:LHALF, 0:HR]
        tabBt = whaug.ap()[LHALF:N_TAB, 0:HR]

        adcomp = ctx.enter_context(tc.tile_pool(name="adcomp", bufs=6))
        adscr = ctx.enter_context(tc.tile_pool(name="adscr", bufs=4))
        selp = ctx.enter_context(tc.tile_pool(name="selp", bufs=6))
        sps = ctx.enter_context(tc.tile_pool(name="sps", bufs=2, space="PSUM"))
        ssb = ctx.enter_context(tc.tile_pool(name="ssb", bufs=3))
        adp = ctx.enter_context(tc.tile_pool(name="adp", bufs=2, space="PSUM"))
        accp = ctx.enter_context(tc.tile_pool(name="accp", bufs=2, space="PSUM"))
        acc2p = ctx.enter_context(tc.tile_pool(name="acc2p", bufs=2, space="PSUM"))
        scp = ctx.enter_context(tc.tile_pool(name="scp", bufs=8))
        smallp = ctx.enter_context(tc.tile_pool(name="smallp", bufs=4))

        # --- software-pipelined slot loop (stages lagged so every engine's
        # in-order queue sees only already-satisfied dependencies) ---
        NG = (J + 7) // 8  # a_dst groups
        ad_tiles = [None] * NG
        m0_t = [None] * J
        sel_t = [None] * J
        adst_t = [None] * J
        eex_t = [None] * J
        acc_t = [None] * J
        acc2_t = [None] * J

        def adgrp(g):
            # a_dst gathers + compaction + A/B blend for slots [8g, 8g+8)
            j = 8 * g
            cA = adcomp.tile([P, 8, K], bf16)
            cB = adcomp.tile([P, 8, K], bf16)
            ad = adcomp.tile([P, 8, K], bf16)
            for tab, idx_sb, dstc in ((tabAt, adA_sb, cA), (tabBt, adB_sb, cB)):
                scr = adscr.tile([P, 8, HR], bf16)
                nc.gpsimd.dma_gather(
                    out_ap=scr[:], in_ap=tab,
                    idxs_ap=idx_sb[:, j * 8 : (j + 8) * 8],
                    num_idxs=8 * P, num_idxs_reg=8 * P,
                    elem_size=HR, elem_step=ROW,
                )
                nc.scalar.copy(out=dstc[:], in_=scr[:, :, 0:K])
            # wrong-half idx point at a zero pad row -> plain add merges A/B
            nc.vector.tensor_tensor(out=ad[:], in0=cA[:], in1=cB[:],
                                    op=mybir.AluOpType.add)
            ad_tiles[g] = ad

        def s0(j):  # gathers + sel
            m0t = m0p.tile([P, NCH, ROW], bf16)
            m0_t[j] = m0t
            for tab, gsb, c0 in ((tabA, gA_sb, 0), (tabB, gB_sb, CPB)):
                nc.gpsimd.dma_gather(
                    out_ap=m0t[:, c0 : c0 + CPB, :], in_ap=tab,
                    idxs_ap=gsb[:, j * 64 : (j + 1) * 64],
                    num_idxs=SEGCAP, num_idxs_reg=SEGCAP,
                    elem_size=ROW, elem_step=ROW,
                )
            sel = selp.tile([P, P, NCH], bf16)
            sel_t[j] = sel
            nc.sync.dma_start(
                out=sel[:],
                in_=sel_d.ap()[:, j * P * NCH : (j + 1) * P * NCH],
            )

        def s1(j):  # per-chunk transposes + a_dst scatter matmuls
            sel = sel_t[j]
            ad = ad_tiles[j // 8]
            adst = adp.tile([P, NCH, K], f32)
            adst_t[j] = adst
            for g0 in range(0, NCH, TGRP):
                s_ps = sps.tile([P, TGRP, P], bf16)
                for q in range(TGRP):
                    nc.tensor.transpose(out=s_ps[:, q, :], in_=sel[:, :, g0 + q],
                                        identity=ident_sb[:])
                s_sb = ssb.tile([P, TGRP, P], bf16)
                nc.scalar.copy(out=s_sb[:], in_=s_ps[:])
                for q in range(TGRP):
                    nc.tensor.matmul(out=adst[:, g0 + q, :], lhsT=s_sb[:, q, :],
                                     rhs=ad[:, j % 8, :], start=True, stop=True)
            # gpsimd cannot read PSUM on real HW: evacuate to SBUF bf16
            # (lossless: values are bf16 a_dst entries scattered by a 0/1 S)
            adstb = scp.tile([P, NCH, K], bf16)
            nc.scalar.copy(out=adstb[:], in_=adst[:])
            adst_t[j] = adstb

        def s2(j):  # e_exp chain + alpha multiply
            m0t = m0_t[j]
            adstb = adst_t[j]
            s_t = scp.tile([P, NCH, K], f32)
            nc.vector.tensor_tensor(out=s_t[:], in0=m0t[:, :, 0:K],
                                    in1=adstb[:], op=mybir.AluOpType.add)
            # leaky on DVE (Act Lrelu would force a 1.3us act-table reload
            # per switch between Lrelu and Exp/Copy)
            nc.vector.scalar_tensor_tensor(
                out=s_t[:], in0=s_t[:], scalar=NEG_SLOPE, in1=s_t[:],
                op0=mybir.AluOpType.mult, op1=mybir.AluOpType.max,
            )
            eex = scp.tile([P, NCH, K], bf16)
            eex_t[j] = eex
            nc.scalar.activation(out=eex[:], in_=s_t[:],
                                 func=mybir.ActivationFunctionType.Exp)
            msg4 = m0t[:, :, 0:OUT_DIM].rearrange("p n (d k) -> p n d k", k=K)
            ee = eex[:]
            nc.vector.tensor_tensor(
                out=msg4, in0=msg4,
                in1=bass.AP(tensor=ee.tensor, offset=ee.offset,
                            ap=[ee.ap[0], [K, NCH], [0, DK], [1, K]]),
                op=mybir.AluOpType.mult,
            )

        def s3(j):  # segment-sum accumulations
            sel = sel_t[j]
            m0t = m0_t[j]
            eex = eex_t[j]
            acc = accp.tile([P, OUT_DIM], f32)
            acc2 = acc2p.tile([P, K], f32)
            acc_t[j] = acc
            acc2_t[j] = acc2
            for ci in range(NCH):
                nc.tensor.matmul(out=acc[:], lhsT=sel[:, :, ci],
                                 rhs=m0t[:, ci, :],
                                 start=(ci == 0), stop=(ci == NCH - 1))
            for ci in range(NCH):
                nc.tensor.matmul(out=acc2[:], lhsT=sel[:, :, ci],
                                 rhs=eex[:, ci, :],
                                 start=(ci == 0), stop=(ci == NCH - 1))

        def s4(j):  # normalize into the resident output tile
            r = smallp.tile([P, K], bf16)
            with nc.allow_low_precision(reason="1/e_sum bf16 scale"):
                nc.vector.reciprocal(out=r[:], in_=acc2_t[j][:])
            accb = smallp.tile([P, OUT_DIM], bf16)
            nc.scalar.copy(out=accb[:], in_=acc_t[j][:])
            nc.vector.tensor_tensor(
                out=outsb[:, j, :], in0=accb[:],
                in1=bass.AP(tensor=r.tensor, offset=r.offset,
                            ap=[r.ap[0], [0, DK], [1, K]]),
                op=mybir.AluOpType.mult,
            )

        WCHUNK = 8
        for jj in range(J + 4):
            if jj < J:
                if jj % 8 == 0 and jj // 8 < NG:
                    adgrp(jj // 8)
                s0(jj)
            if 0 <= jj - 1 < J:
                s1(jj - 1)
            if 0 <= jj - 2 < J:
                s2(jj - 2)
            if 0 <= jj - 3 < J:
                s3(jj - 3)
            if 0 <= jj - 4 < J:
                s4(jj - 4)
                if (jj - 3) % WCHUNK == 0 and 0 < jj - 3 < J:
                    lo = jj - 3 - WCHUNK
                    nc.sync.dma_start(
                        out=out_d.ap().rearrange("(j p) f -> p j f", p=BLK)[
                            :, lo : jj - 3, :
                        ],
                        in_=outsb[:, lo : jj - 3, :],
                    )
        lo = ((J - 1) // WCHUNK) * WCHUNK
        nc.sync.dma_start(
            out=out_d.ap().rearrange("(j p) f -> p j f", p=BLK)[:, lo:J, :],
            in_=outsb[:, lo:J, :],
        )

    nc.compile()
    return nc


def run(h, edge_src, edge_dst, W, attn, n_cores=N_CORES, trace=False):
    from concourse.bass_utils import run_bass_kernel_spmd

    n_nodes = h.shape[0]
    h = np.asarray(h, dtype=np.float32)
    W = np.asarray(W, dtype=np.float32)
    attn = np.asarray(attn, dtype=np.float32)
    edge_src = np.asarray(edge_src)
    edge_dst = np.asarray(edge_dst)

    plan = build_plan(edge_src, edge_dst, n_nodes, n_cores)
    # global permuted layout, then per-core compacted column selections
    hTd = np.zeros((IN_DIM, N_PAD), dtype=BF16)
    hTd[:, plan["posof"]] = h.T.astype(BF16)[:, np.arange(n_nodes)]
    hT_c = np.zeros((n_cores, IN_DIM, N_TAB), dtype=BF16)
    for cix in range(n_cores):
        cols = plan["cols"][cix]
        m = cols >= 0
        hT_c[cix][:, m] = hTd[:, cols[m]]
    # per-head rotation T_k with row 0 == attn_k
    T = np.zeros((K, DK, DK), np.float64)
    Tinv = np.zeros((K, DK, DK), np.float64)
    for k in range(K):
        M = np.concatenate([attn[k][:, None].astype(np.float64),
                            np.eye(DK)], axis=1)
        Q, R = np.linalg.qr(M)
        Tk = Q.T.copy()
        Tk[0] *= R[0, 0]  # row 0 becomes exactly attn_k
        T[k] = Tk
        Tinv[k] = np.linalg.inv(Tk)
    Wv = np.einsum("kde,kei->dki", T, W.reshape(K, DK, IN_DIM).astype(np.float64))
    waugT = Wv.reshape(OUT_DIM, IN_DIM).T.astype(BF16)
    ident = np.eye(P, dtype=BF16)

    nc = build_program(n_cores)

    in_maps = []
    for cix in range(n_cores):
        in_maps.append({
            "hT": hT_c[cix],
            "waugT": waugT,
            "gA": plan["gA"][cix],
            "gB": plan["gB"][cix],
            "adA": plan["adA"][cix],
            "adB": plan["adB"][cix],
            "seld": plan["sel"][cix],
            "ident": ident,
        })
    try:
        res = run_bass_kernel_spmd(nc, in_maps, list(range(n_cores)), trace=trace)
    except Exception:
        if not trace:
            raise
        res = run_bass_kernel_spmd(nc, in_maps, list(range(n_cores)), trace=False)

    bins = plan["bins"]
    vout = np.zeros((n_nodes, OUT_DIM), dtype=np.float32)
    for cix in range(n_cores):
        o = np.asarray(res.results[cix]["out"], dtype=np.float32)
        for j in range(J):
            lst = bins[cix * J + j]
            if lst:
                vout[np.asarray(lst)] = o[j * BLK : j * BLK + len(lst)]
    # undo the (d,k) column interleave and the per-head rotation
    v = vout.reshape(n_nodes, DK, K).transpose(0, 2, 1)
    out = np.einsum("ked,nkd->nke", Tinv, v.astype(np.float64)).astype(np.float32)
    return out, res


def kernel(h, edge_src, edge_dst, W, attn):
    out, _ = run(h, edge_src, edge_dst, W, attn)
    return out
